# revision 10
# baseline (speedup 1.0000x reference)
"""CZ gate on a batch of state vectors, data-parallel across 8 NeuronCores.

out[b, i] = state[b, i] * (-1 if bits (nq-1-control) and (nq-1-target) of
basis index i are both set else +1). For the graded instance
(control=0, target=1, num_qubits=13, D=8192) the diagonal is +1 on
columns [0, 6144) and -1 on columns [6144, 8192).

Strategy:
  - The full state ships to the device once as the DONATED output buffer.
    XLA aliases the donated input buffer onto the NEFF output, and the
    NEFF does not zero its outputs, so every +1 column passes through
    untouched. Only the -1 columns are processed on-device: load to SBUF,
    flip the sign on VectorE, store back in place. That is 32 MiB of
    DMA traffic per core instead of the 128 MiB a full read+write kernel
    would move (4x), and it is the information-theoretic minimum for the
    device work.
  - The per-core program is raw bacc (no Tile scheduler): loads issue on
    the SP HWDGE queue with a per-chunk semaphore, VectorE negates each
    chunk in place, stores issue on the ACT HWDGE queue, and SP finally
    waits for all store bytes to land and clears the semaphores so the
    loaded NEFF can be re-executed. Chunk sizes are graded (small at both
    ends) to shorten pipeline fill and drain.
  - Batch rows are sharded 8-way with shard_map; the jitted executable is
    cached so repeat calls skip compilation.

Measured on trn2 (NTFF profile, core 0): ~90 us when the HBM-stack
neighbor core is staggered (SBUF-fabric bound, ~405 GB/s of the 436
ceiling), ~105 us when truly concurrent (HBM bound, 32 MiB at ~358 GB/s
per core + fixed ~7 us preamble).
"""

import os
import sys
import types

import numpy as np

# concourse's trace path imports antenv.axon_hooks unconditionally when
# BASS_TRACE is set; this container's antenv lacks that submodule. Register
# a no-op fallback so a stray BASS_TRACE can never crash the kernel. Test
# harnesses install the real hook before importing this module.
try:
    import antenv.axon_hooks  # noqa: F401
except ImportError:
    import antenv

    _hook_holder = [None]
    _axon_hooks = types.ModuleType("antenv.axon_hooks")
    _axon_hooks.set_axon_ntff_profile_hook = (
        lambda h: _hook_holder.__setitem__(0, h)
    )
    _axon_hooks.get_axon_ntff_profile_hook = lambda: _hook_holder[0]
    sys.modules["antenv.axon_hooks"] = _axon_hooks
    antenv.axon_hooks = _axon_hooks

import concourse.bacc as bacc
from concourse import mybir

BATCH = 16384
D = 8192
N_CORES = 8
ROWS = BATCH // N_CORES  # 2048 rows per core
P = 128                  # SBUF partitions

# Rows-per-partition per pipeline chunk (sums to ROWS // P = 16). Small
# chunks at both ends shorten pipeline fill (first negate starts sooner)
# and drain (last store is short).
KLIST = (1, 1, 2, 4, 4, 2, 1, 1)

LAST_EXEC_TIME_NS = None
LAST_RESULT = None

_CACHE = {}


def _mask_runs(neg_mask):
    """Maximal runs of -1 columns, as ((start, end), ...)."""
    neg_runs = []
    start = 0
    for i in range(1, D + 1):
        if i == D or neg_mask[i] != neg_mask[start]:
            if neg_mask[start]:
                neg_runs.append((start, i))
            start = i
    return tuple(neg_runs)


def _build_program(neg_runs):
    """Raw-bacc program: three engines, minimal sync.

    Per (run, chunk): SP issues the load DMA (then_inc per-chunk in-sem),
    DVE waits that sem and negates the tile in place (inc dve-sem), ACT
    waits the dve-sem and issues the store DMA (then_inc shared out-sem).
    SP finally waits for all store bytes to land and clears the sems so
    the loaded NEFF can be re-executed.
    """
    nc = bacc.Bacc("TRN2", target_bir_lowering=False, debug=False)
    y = nc.dram_tensor(
        "y", [ROWS, D], mybir.dt.float32, kind="ExternalOutput"
    ).ap()

    assert sum(KLIST) == ROWS // P
    chunks = []  # (dram_view, sbuf_tile_ap) per chunk across all runs
    for ri, (s, e) in enumerate(neg_runs):
        w = e - s
        r0 = 0
        for c, k in enumerate(KLIST):
            rows = P * k
            view = y[r0:r0 + rows, s:e].rearrange("(p k) d -> p k d", k=k)
            t = nc.alloc_sbuf_tensor(f"t_{ri}_{c}", [P, k, w], mybir.dt.float32)
            chunks.append((view, t.ap()))
            r0 += rows

    n = len(chunks)
    in_sems = [nc.alloc_semaphore(f"in{i}") for i in range(n)]
    dve_sem = nc.alloc_semaphore("dve")
    out_sem = nc.alloc_semaphore("outs")

    for i, (view, t) in enumerate(chunks):
        nc.sync.dma_start(out=t[:], in_=view).then_inc(in_sems[i], 16)
    for i, (view, t) in enumerate(chunks):
        nc.vector.wait_ge(in_sems[i], 16)
        nc.vector.tensor_scalar_mul(t[:], t[:], -1.0).then_inc(dve_sem, 1)
    for i, (view, t) in enumerate(chunks):
        nc.scalar.wait_ge(dve_sem, i + 1)
        nc.scalar.dma_start(out=view, in_=t[:]).then_inc(out_sem, 16)

    # All store bytes confirmed landed, then reset sems for re-execution.
    nc.sync.wait_ge(out_sem, 16 * n)
    for sem in [*in_sems, dve_sem, out_sem]:
        nc.sync.sem_clear(sem)

    nc.compile()
    return nc


def _get_exec(neg_runs):
    """(once per mask) build + compile the program and jit the 8-core runner."""
    if neg_runs in _CACHE:
        return _CACHE[neg_runs]

    import jax
    from jax.experimental.shard_map import shard_map
    from jax.sharding import Mesh, PartitionSpec

    from concourse.bass2jax import (
        _bass_exec_p,
        install_neuronx_cc_hook,
        partition_id_tensor,
    )

    nc = _build_program(neg_runs)
    install_neuronx_cc_hook()

    partition_name = (
        nc.partition_id_tensor.name if nc.partition_id_tensor else None
    )
    out_aval = jax.core.ShapedArray((ROWS, D), np.float32)
    all_in_names = ["y"] + ([partition_name] if partition_name else [])

    def _body(*args):
        operands = list(args)
        if partition_name is not None:
            operands.append(partition_id_tensor())
        outs = _bass_exec_p.bind(
            *operands,
            out_avals=(out_aval,),
            in_names=tuple(all_in_names),
            out_names=("y",),
            lowering_input_output_aliases=(),
            sim_require_finite=True,
            sim_require_nnan=True,
            nc=nc,
        )
        return tuple(outs)

    devices = jax.devices()[:N_CORES]
    mesh = Mesh(np.asarray(devices), ("core",))
    sharded = jax.jit(
        shard_map(
            _body,
            mesh=mesh,
            in_specs=(PartitionSpec("core"),),
            out_specs=(PartitionSpec("core"),),
            check_rep=False,
        ),
        donate_argnums=(0,),
        keep_unused=True,
    )
    _CACHE[neg_runs] = (nc, sharded)
    return nc, sharded


def _trace_requested():
    v = os.environ.get("BASS_TRACE", "")
    return v not in ("", "0", "false", "False")


def _run_traced(nc, exec_fn):
    """Wrap one execution with NTFF capture; mirrors run_bass_kernel_spmd's
    axon trace branch. Returns (outputs, exec_time_ns, results_obj)."""
    import glob as globmod
    import tempfile

    from antenv.axon_hooks import get_axon_ntff_profile_hook

    import gauge.profiler
    from concourse.bass_utils import (
        FishPath,
        _process_ntff_profile,
        upload_artifacts,
    )

    hook = get_axon_ntff_profile_hook()
    if hook is None:
        return exec_fn(), None, None

    neff_dir = tempfile.mkdtemp()
    with hook(neff_dir, [0]):
        out = exec_fn()
    try:
        ntffs = globmod.glob(os.path.join(neff_dir, "*_body*.ntff"))
        if not ntffs:
            return out, None, None
        sharepath = upload_artifacts(neff_dir)
        profile = gauge.profiler.Profile(
            profile_path=FishPath(neff_dir),
            kernel_dev_mode=True,
            profile_on_exit=False,
            bass_kernel=nc.m,
            offline_processing=True,
            fname="*_body*",
            metadata={"artifacts_path": sharepath},
        )
        res = _process_ntff_profile(
            profile, neff_dir, nc, list(range(N_CORES)), None, False, {},
            trace_events=False,
        )
        return out, res.exec_time_ns, res
    except Exception as e:
        print(f"NTFF post-processing failed: {e}", file=sys.stderr)
        return out, None, None


def kernel(state, control, target, num_qubits):
    global LAST_EXEC_TIME_NS, LAST_RESULT
    state = np.asarray(state)
    control = int(np.asarray(control))
    target = int(np.asarray(target))
    nq = int(np.asarray(num_qubits))
    assert state.shape == (BATCH, D), state.shape

    c2 = nq - control - 1
    t2 = nq - target - 1
    idx = np.arange(D)
    neg_mask = (((idx >> c2) & 1) != 0) & (((idx >> t2) & 1) != 0)
    neg_runs = _mask_runs(neg_mask)

    out_dtype = state.dtype
    state_f32 = np.ascontiguousarray(state, dtype=np.float32)
    if not neg_runs:
        return state_f32.copy().astype(out_dtype, copy=False)

    nc, sharded = _get_exec(neg_runs)

    # `state_f32` is donated: its device buffer becomes the NEFF output
    # buffer, so untouched columns pass through. The host array is
    # unaffected (jax copies host->device before donating).
    run = lambda: np.asarray(sharded(state_f32)[0])

    if _trace_requested():
        out, exec_ns, res = _run_traced(nc, run)
        LAST_EXEC_TIME_NS = exec_ns
        LAST_RESULT = res
    else:
        out = run()
        LAST_EXEC_TIME_NS = None
        LAST_RESULT = None
    return out.astype(out_dtype, copy=False)


# revision 12
# speedup vs baseline: 1.0274x; 1.0274x over previous
"""CZ gate on a batch of state vectors, data-parallel across 8 NeuronCores.

out[b, i] = state[b, i] * (-1 if bits (nq-1-control) and (nq-1-target) of
basis index i are both set else +1). For the graded instance
(control=0, target=1, num_qubits=13, D=8192) the diagonal is +1 on
columns [0, 6144) and -1 on columns [6144, 8192).

Strategy:
  - The full state ships to the device once as the DONATED output buffer.
    XLA aliases the donated input buffer onto the NEFF output, and the
    NEFF does not zero its outputs, so every +1 column passes through
    untouched. Only the -1 columns are processed on-device: load to SBUF,
    flip the sign on VectorE, store back in place. That is 32 MiB of
    DMA traffic per core instead of the 128 MiB a full read+write kernel
    would move (4x), and it is the information-theoretic minimum for the
    device work.
  - The per-core program is raw bacc (no Tile scheduler): loads issue on
    the SP HWDGE queue with a per-chunk semaphore, VectorE negates each
    chunk in place, stores issue on the ACT HWDGE queue, and SP finally
    waits for all store bytes to land and clears the semaphores so the
    loaded NEFF can be re-executed. Chunk sizes are graded (small at both
    ends) to shorten pipeline fill and drain.
  - Batch rows are sharded 8-way with shard_map; the jitted executable is
    cached so repeat calls skip compilation.

Measured on trn2 (NTFF profile, core 0): ~90 us when the HBM-stack
neighbor core is staggered (SBUF-fabric bound, ~405 GB/s of the 436
ceiling), ~105 us when truly concurrent (HBM bound, 32 MiB at ~358 GB/s
per core + fixed ~7 us preamble).
"""

import os
import sys
import types

import numpy as np

# concourse's trace path imports antenv.axon_hooks unconditionally when
# BASS_TRACE is set; this container's antenv lacks that submodule. Register
# a no-op fallback so a stray BASS_TRACE can never crash the kernel. Test
# harnesses install the real hook before importing this module.
try:
    import antenv.axon_hooks  # noqa: F401
except ImportError:
    import antenv

    _hook_holder = [None]
    _axon_hooks = types.ModuleType("antenv.axon_hooks")
    _axon_hooks.set_axon_ntff_profile_hook = (
        lambda h: _hook_holder.__setitem__(0, h)
    )
    _axon_hooks.get_axon_ntff_profile_hook = lambda: _hook_holder[0]
    sys.modules["antenv.axon_hooks"] = _axon_hooks
    antenv.axon_hooks = _axon_hooks

import concourse.bacc as bacc
from concourse import mybir

BATCH = 16384
D = 8192
N_CORES = 8
ROWS = BATCH // N_CORES  # 2048 rows per core
P = 128                  # SBUF partitions

# Rows-per-partition per pipeline chunk (sums to ROWS // P = 16). Small
# chunks at both ends shorten pipeline fill (first negate starts sooner)
# and drain (last store is short).
KLIST = (1, 1, 2, 4, 4, 2, 1, 1)

LAST_EXEC_TIME_NS = None
LAST_RESULT = None

_CACHE = {}


def _mask_runs(neg_mask):
    """Maximal runs of -1 columns, as ((start, end), ...)."""
    neg_runs = []
    start = 0
    for i in range(1, D + 1):
        if i == D or neg_mask[i] != neg_mask[start]:
            if neg_mask[start]:
                neg_runs.append((start, i))
            start = i
    return tuple(neg_runs)


def _build_program(neg_runs):
    """Raw-bacc program: three engines, minimal sync.

    Per (run, chunk): SP issues the load DMA (then_inc per-chunk in-sem),
    DVE waits that sem and negates the tile in place (inc dve-sem), ACT
    waits the dve-sem and issues the store DMA (then_inc shared out-sem).
    SP finally waits for all store bytes to land and clears the sems so
    the loaded NEFF can be re-executed.
    """
    nc = bacc.Bacc("TRN2", target_bir_lowering=False, debug=False)
    y = nc.dram_tensor(
        "y", [ROWS, D], mybir.dt.float32, kind="ExternalOutput"
    ).ap()

    assert sum(KLIST) == ROWS // P
    chunks = []  # (dram_view, sbuf_tile_ap) per chunk across all runs
    for ri, (s, e) in enumerate(neg_runs):
        w = e - s
        r0 = 0
        for c, k in enumerate(KLIST):
            rows = P * k
            view = y[r0:r0 + rows, s:e].rearrange("(p k) d -> p k d", k=k)
            t = nc.alloc_sbuf_tensor(f"t_{ri}_{c}", [P, k, w], mybir.dt.float32)
            chunks.append((view, t.ap()))
            r0 += rows

    n = len(chunks)
    in_sems = [nc.alloc_semaphore(f"in{i}") for i in range(n)]
    dve_sem = nc.alloc_semaphore("dve")
    out_sem = nc.alloc_semaphore("outs")

    for i, (view, t) in enumerate(chunks):
        nc.sync.dma_start(out=t[:], in_=view).then_inc(in_sems[i], 16)
    for i, (view, t) in enumerate(chunks):
        nc.vector.wait_ge(in_sems[i], 16)
        nc.vector.tensor_scalar_mul(t[:], t[:], -1.0).then_inc(dve_sem, 1)
    for i, (view, t) in enumerate(chunks):
        nc.scalar.wait_ge(dve_sem, i + 1)
        nc.scalar.dma_start(out=view, in_=t[:]).then_inc(out_sem, 16)

    # All store bytes confirmed landed, then reset sems for re-execution.
    nc.sync.wait_ge(out_sem, 16 * n)
    for sem in [*in_sems, dve_sem, out_sem]:
        nc.sync.sem_clear(sem)

    nc.compile()
    return nc


def _get_exec(neg_runs):
    """(once per mask) build + compile the program and jit the 8-core runner."""
    if neg_runs in _CACHE:
        return _CACHE[neg_runs]

    import jax
    from jax.experimental.shard_map import shard_map
    from jax.sharding import Mesh, PartitionSpec

    from concourse.bass2jax import (
        _bass_exec_p,
        install_neuronx_cc_hook,
        partition_id_tensor,
    )

    nc = _build_program(neg_runs)
    install_neuronx_cc_hook()

    partition_name = (
        nc.partition_id_tensor.name if nc.partition_id_tensor else None
    )
    out_aval = jax.core.ShapedArray((ROWS, D), np.float32)
    all_in_names = ["y"] + ([partition_name] if partition_name else [])

    def _body(*args):
        operands = list(args)
        if partition_name is not None:
            operands.append(partition_id_tensor())
        outs = _bass_exec_p.bind(
            *operands,
            out_avals=(out_aval,),
            in_names=tuple(all_in_names),
            out_names=("y",),
            lowering_input_output_aliases=(),
            sim_require_finite=True,
            sim_require_nnan=True,
            nc=nc,
        )
        return tuple(outs)

    devices = jax.devices()[:N_CORES]
    mesh = Mesh(np.asarray(devices), ("core",))
    sharded = jax.jit(
        shard_map(
            _body,
            mesh=mesh,
            in_specs=(PartitionSpec("core"),),
            out_specs=(PartitionSpec("core"),),
            check_rep=False,
        ),
        donate_argnums=(0,),
        keep_unused=True,
    )
    _CACHE[neg_runs] = (nc, sharded)
    return nc, sharded


def _trace_requested():
    v = os.environ.get("BASS_TRACE", "")
    return v not in ("", "0", "false", "False")


def _run_traced(nc, exec_fn):
    """Wrap one execution with NTFF capture; mirrors run_bass_kernel_spmd's
    axon trace branch. Returns (outputs, exec_time_ns, results_obj)."""
    import glob as globmod
    import tempfile

    from antenv.axon_hooks import get_axon_ntff_profile_hook

    import gauge.profiler
    from concourse.bass_utils import (
        FishPath,
        _process_ntff_profile,
        upload_artifacts,
    )

    hook = get_axon_ntff_profile_hook()
    if hook is None:
        return exec_fn(), None, None

    neff_dir = tempfile.mkdtemp()
    with hook(neff_dir, [0]):
        out = exec_fn()
    try:
        ntffs = globmod.glob(os.path.join(neff_dir, "*_body*.ntff"))
        if not ntffs:
            return out, None, None
        sharepath = upload_artifacts(neff_dir)
        profile = gauge.profiler.Profile(
            profile_path=FishPath(neff_dir),
            kernel_dev_mode=True,
            profile_on_exit=False,
            bass_kernel=nc.m,
            offline_processing=True,
            fname="*_body*",
            metadata={"artifacts_path": sharepath},
        )
        res = _process_ntff_profile(
            profile, neff_dir, nc, list(range(N_CORES)), None, False, {},
            trace_events=False,
        )
        return out, res.exec_time_ns, res
    except Exception as e:
        print(f"NTFF post-processing failed: {e}", file=sys.stderr)
        return out, None, None


def kernel(state, control, target, num_qubits):
    global LAST_EXEC_TIME_NS, LAST_RESULT
    state = np.asarray(state)
    control = int(np.asarray(control))
    target = int(np.asarray(target))
    nq = int(np.asarray(num_qubits))
    assert state.shape == (BATCH, D), state.shape

    c2 = nq - control - 1
    t2 = nq - target - 1
    idx = np.arange(D)
    neg_mask = (((idx >> c2) & 1) != 0) & (((idx >> t2) & 1) != 0)
    neg_runs = _mask_runs(neg_mask)

    out_dtype = state.dtype
    state_f32 = np.ascontiguousarray(state, dtype=np.float32)
    if not neg_runs:
        return state_f32.copy().astype(out_dtype, copy=False)

    nc, sharded = _get_exec(neg_runs)

    # `state_f32` is donated: its device buffer becomes the NEFF output
    # buffer, so untouched columns pass through. The host array is
    # unaffected (jax copies host->device before donating).
    run = lambda: np.asarray(sharded(state_f32)[0])

    if _trace_requested():
        out, exec_ns, res = _run_traced(nc, run)
        LAST_EXEC_TIME_NS = exec_ns
        LAST_RESULT = res
    else:
        out = run()
        LAST_EXEC_TIME_NS = None
        LAST_RESULT = None
    return out.astype(out_dtype, copy=False)
